# revision 41
# baseline (speedup 1.0000x reference)
"""BDC loss kernel for 8 Trainium2 NeuronCores.

reference:
    intra = mean over rows of ||f - c_l||^2 / exp(cos(f, c_l))
    adv   = sum over label-differing ordered pairs of relu(0.5 - cos_sim(f_i, f_j)) / n_pairs
    out   = intra + 0.5 * adv

Strategy (SPMD, one program on 8 cores, per-core data differs):
  - The B x B cosine-sim hinge sum is symmetric; each unordered tile-pair is
    computed once using a circulant assignment over the 64 row-tiles of 128:
    global row-tile A computes col-tiles at distance d = 0..32 (mod 64), i.e.
    a 4224-column span starting at its own diagonal. The span is processed as
    11 uniform 384-column fp8 DoubleRow matmul chunks (4224 = 11*384), so the
    PE stream has no narrow LDWEIGHTS-bound groups.
  - Core c owns global row-tiles 8c..8c+7 and receives features rows rolled
    by 1024*c, truncated to the 5120 rows the core ever touches.
  - All O(B*D) prep runs on the host: rows are sorted by label, normalized
    (exact f64 norms), transposed to K-major, and cast to fp8. The device
    receives matmul-ready operands, so its PE stream is matmuls only.
  - Hinge eviction with row-sum accumulation is load-balanced across DVE and
    ACT (GPSIMD cannot access PSUM). ACT slots hold +hinge (Relu with
    scale=-1), DVE slots hold -hinge (min(sim - margin, 0)); the host flips
    signs per-slot.
  - Inputs are host-sorted by label, so same-label pairs live within ~30
    rows of the diagonal: chunk sums need no mask; two narrow is_equal
    corrections per row-tile (subtracted on the host) fix up the strip
    [t*128, t*128+256) where same-label pairs can occur.
  - The intra term needs only q = sum((f+cb)^2) per row on the device (ACT
    Square with accumulate on a host-prepared bf16 f+cb array); with
    host-known h1 = ||f||^2+||cb||^2 and rp = 1/(||f||*||cb||):
    sq_err = 2*h1 - q and sim = q*rp/2 - h1*rp/2 are linear in q.
  - Host does the final tiny reduction in float64 (exact at fp32 scale).
"""

import numpy as np

B, D, C = 8192, 1024, 1000
NCORES = 8
SHARD = B // NCORES            # 1024 rows owned per core
RT = SHARD // 128              # 8 row-tiles per core
NTILES = B // 128              # 64 global row-tiles
DMAX = 32                      # circulant distance range 0..32
LROWS = (RT + DMAX) * 128      # 5120 local rows each core needs
KT = D // 128                  # 8 K-chunks
CW = 384                       # matmul chunk width
NCH = 11                       # chunks per row-tile (11*384 = 4224 cols)
SLOTS = 15                     # accum slots per row-tile (13 dist + 2 corr)
LABCOLS = (RT + 1) * 128       # 1152 label columns needed for corrections
NVEC = 3 * RT                  # packed per-row scalars (h1x2, r2, hr)
ALPHA, LAMBDA_ADV, MARGIN, EPS = 1.0, 0.5, 0.5, 1e-8

_CACHE = {}


def _chunk_colend(tc_pair):
    t, j = tc_pair
    return t * 128 + (j + 1) * CW


# Distance weights per slot: chunk j covers diag-offsets [384j, 384j+384) =
# d-tiles 3j..3j+2. d=0 and d=32 are computed from both sides (weight 1);
# d=1..31 from one side only (weight 2). Slots:
#   0: j0 [0:128]    d=0   w=1        1: j0 [128:384]  d=1,2  w=2
#   2..10: j1..j9    full  w=2
#   11: j10 [0:256]  d=30,31 w=2      12: j10 [256:384] d=32  w=1
#   13: corr [0:128] vs slot 0   w=-1
#   14: corr [128:256] vs slot 1 w=-2
W_SLOT = [1.0] + [2.0] * 11 + [1.0, -1.0, -2.0]

# Static eviction-engine assignment, shared by device build and host
# combine. GPSIMD cannot access PSUM, so evictions split across DVE ('v')
# and ACT ('a'). j==0 stays on DVE because the same-label corrections read
# its eviction output tile. ACT also runs the 8 intra squares and is slower
# per eviction (accumulator readout), so it takes a minority of the RR
# chunks.
N_ACT_RR = 34
N_RR = RT * (NCH - 1)          # 80 round-robin-eligible groups
N_TAIL_V = 0                   # last RR groups pinned to DVE (tried 3: DVE
                               # lagged late-stream and stalled PE on PSUM)


def _engine_plan():
    pend = sorted([(t, j) for t in range(RT) for j in range(NCH)],
                  key=_chunk_colend)
    plan = {}
    i = 0
    nrr = N_RR - N_TAIL_V
    for t, j in pend:
        if j == 0:
            plan[(t, j)] = "v"
        elif i >= nrr:
            plan[(t, j)] = "v"
            i += 1
        else:
            a = ((i + 1) * N_ACT_RR) // nrr > (i * N_ACT_RR) // nrr
            plan[(t, j)] = "a" if a else "v"
            i += 1
    return pend, plan


PEND, ENG = _engine_plan()


def _build():
    import concourse.bass as bass
    import concourse.tile as tile
    from concourse import bacc, mybir

    f32 = mybir.dt.float32
    f16 = mybir.dt.float16
    bf16 = mybir.dt.bfloat16
    f8 = mybir.dt.float8e4

    nc = bacc.Bacc("TRN2", target_bir_lowering=False, debug=False,
                   num_devices=NCORES)

    fhat_dram = nc.dram_tensor("fhat_t", [128, KT * LROWS], f8,
                               kind="ExternalInput")
    s_dram = nc.dram_tensor("s_in", [SHARD, D], bf16, kind="ExternalInput")
    lab_dram = nc.dram_tensor("lab_f16", [LABCOLS], f16, kind="ExternalInput")
    vec_dram = nc.dram_tensor("vecs", [128, NVEC + RT], f32,
                              kind="ExternalInput")
    adv_dram = nc.dram_tensor("adv_out", [128, RT * SLOTS], f32,
                              kind="ExternalOutput")
    intra_dram = nc.dram_tensor("intra_out", [128, RT], f32,
                                kind="ExternalOutput")

    with tile.TileContext(nc) as tc:
        from contextlib import ExitStack
        with ExitStack() as ctx:
            # pool-buffer count is kept minimal: TileContext teardown costs
            # ~3 serialized semaphore ops per buffer per engine (~0.5us each)
            singles = ctx.enter_context(tc.tile_pool(name="singles", bufs=1))
            wv = ctx.enter_context(tc.tile_pool(name="wv", bufs=2))
            wa = ctx.enter_context(tc.tile_pool(name="wa", bufs=2))
            wc = ctx.enter_context(tc.tile_pool(name="wc", bufs=1))
            dsc = ctx.enter_context(tc.tile_pool(name="dsc", bufs=1))
            psum_mm = ctx.enter_context(
                tc.tile_pool(name="psum_mm", bufs=8,
                             space=bass.MemorySpace.PSUM))

            # ---- persistent tiles ----
            fhatT = singles.tile([128, KT, LROWS], f8)      # K-major fhat
            s_all = singles.tile([128, RT, D], bf16)        # f + cb, own rows
            labcol = singles.tile([128, LABCOLS], f16)
            vecs = singles.tile([128, NVEC + RT], f32)  # h1x2|r2|hr|labrow
            adv_acc = singles.tile([128, RT * SLOTS], f32)
            q_t = singles.tile([128, RT], f32)
            sqerr_t = singles.tile([128, RT], f32)
            sim_t = singles.tile([128, RT], f32)
            exp_t = singles.tile([128, RT], f32)
            intra_acc = singles.tile([128, RT], f32)
            zeros = singles.tile([128, CW], f32)
            margin_sb = singles.tile([128, 1], f32)
            warm = singles.tile([128, 1], f32)

            # prime the ACT function table (relu/exp/square share one set)
            nc.vector.memset(warm[:], 1.0)
            nc.vector.memset(margin_sb[:], MARGIN)
            nc.scalar.activation(out=warm[:], in_=warm[:],
                                 func=mybir.ActivationFunctionType.Relu,
                                 bias=margin_sb[:])
            nc.vector.memset(zeros[:], 0.0)

            fhat3 = fhat_dram.ap().rearrange("p (k c) -> p k c", k=KT)

            # ---- all DMA issues up front (the Sync engine serializes
            # descriptor generation; everything here is issued within the
            # first ~10us and lands well before it is needed) ----
            cuts = [0, 512, 1024, 2048, 3072, 4096, LROWS]
            # DMA descriptor generation is serialized per issuing engine, so
            # spread the issues: Sync (SP) takes the fhatT column stream,
            # gpsimd (software DGE, otherwise idle) takes the first k-half
            # and s_in, ACT (second HWDGE queue) takes labcol/vecs.
            # The (0,0) matmul group accumulates k2=0,1 first, so it can
            # start as soon as gpsimd's k-half of the first block lands.
            nc.gpsimd.dma_start(out=fhatT[:, 0:4, cuts[0]:cuts[1]],
                                in_=fhat3[:, 0:4, cuts[0]:cuts[1]])
            nc.sync.dma_start(out=fhatT[:, 4:KT, cuts[0]:cuts[1]],
                              in_=fhat3[:, 4:KT, cuts[0]:cuts[1]])
            nc.sync.dma_start(out=fhatT[:, :, cuts[1]:cuts[2]],
                              in_=fhat3[:, :, cuts[1]:cuts[2]])
            lab_bcast = bass.AP(tensor=lab_dram, offset=0,
                                ap=[[0, 128], [1, LABCOLS]])
            nc.scalar.dma_start(out=labcol[:], in_=lab_bcast)
            nc.scalar.dma_start(out=vecs[:], in_=vec_dram.ap())
            nc.gpsimd.dma_start(
                out=s_all[:],
                in_=s_dram.ap().rearrange("(t p) d -> p t d", p=128))
            for n in range(2, 6):
                nc.sync.dma_start(out=fhatT[:, :, cuts[n]:cuts[n + 1]],
                                  in_=fhat3[:, :, cuts[n]:cuts[n + 1]])
            # NOTE: PE warmup matmuls were tried twice and HURT both times:
            # dummies run at low clock, and any >=1us idle gap before the
            # first real matmul drops the clock to its lowest p-state
            # (idle-fresh PE starts at mid). Leave the PE idle here.

            # ---- one adversarial chunk: 4 DR matmuls + eviction ----
            def emit_chunk(t, j):
                base = t * SLOTS
                c0 = t * 128 + j * CW
                mm = psum_mm.tile([128, CW], f32)
                for k2 in range(KT // 2):
                    nc.tensor.matmul(
                        out=mm[:],
                        lhsT=fhatT[:, 2 * k2:2 * k2 + 2,
                                   t * 128:(t + 1) * 128],
                        rhs=fhatT[:, 2 * k2:2 * k2 + 2, c0:c0 + CW],
                        perf_mode=mybir.MatmulPerfMode.DoubleRow,
                        start=(k2 == 0), stop=(k2 == KT // 2 - 1))
                eng = ENG[(t, j)]
                if j == 0:
                    spans = [(0, 128, base + 0), (128, CW, base + 1)]
                elif j < NCH - 1:
                    spans = [(0, CW, base + 1 + j)]
                else:
                    spans = [(0, 256, base + 11), (256, CW, base + 12)]
                if eng == "a":
                    # +hinge: relu(-sim + margin), row-summed into the slot
                    negh = wa.tile([128, CW], f16, tag="wa")
                    for lo, hi, slot in spans:
                        nc.scalar.activation(
                            out=negh[:, lo:hi], in_=mm[:, lo:hi],
                            func=mybir.ActivationFunctionType.Relu,
                            scale=-1.0, bias=margin_sb[:],
                            accum_out=adv_acc[:, slot:slot + 1])
                else:
                    # -hinge: min(sim - margin, 0)
                    negh = wv.tile([128, CW], f16, tag="wv")
                    for lo, hi, slot in spans:
                        nc.vector.scalar_tensor_tensor(
                            out=negh[:, lo:hi], in0=mm[:, lo:hi],
                            scalar=-MARGIN, in1=zeros[:, lo:hi],
                            op0=mybir.AluOpType.add,
                            op1=mybir.AluOpType.min,
                            accum_out=adv_acc[:, slot:slot + 1])
                if j == 0:
                    # same-label corrections on the strip [t*128, t*128+256)
                    # (labels are host-sorted, so same-label pairs live
                    # there); negh is DVE min-form by construction.
                    for lo, slot in ((0, base + 13), (128, base + 14)):
                        scr = wc.tile([128, 128], f16, tag="corr")
                        nc.vector.scalar_tensor_tensor(
                            out=scr[:], in0=labcol[:, c0 + lo:c0 + lo + 128],
                            scalar=vecs[:, NVEC + t:NVEC + t + 1],
                            in1=negh[:, lo:lo + 128],
                            op0=mybir.AluOpType.is_equal,
                            op1=mybir.AluOpType.mult,
                            accum_out=adv_acc[:, slot:slot + 1])

            # intra: q[p, t] = sum_d (f + cb)^2 on ACT
            def emit_sq(t):
                scr = dsc.tile([128, D], bf16, tag="dsc")
                nc.scalar.activation(
                    out=scr[:], in_=s_all[:, t, :],
                    func=mybir.ActivationFunctionType.Square,
                    accum_out=q_t[:, t:t + 1])

            pend3 = list(range(RT))
            for i, (t, j) in enumerate(PEND):
                emit_chunk(t, j)
                if i >= 24 and i % 6 == 5 and pend3:
                    emit_sq(pend3.pop(0))
            for t in pend3:
                emit_sq(t)

            # ---- final per-row chain (tiny [128, RT] ops). Emitted after
            # ALL evictions: engines execute their queues in order, so a
            # cross-engine-dependent op emitted mid-stream would block the
            # remaining evictions behind it (measured: +11us tail).
            # with q = sum (f+cb)^2 and h1 = ||f||^2 + ||cb||^2:
            #   sq_err = 2 h1 - q ;  sim = q*rp/2 - h1*rp/2
            h1x2 = vecs[:, 0:RT]
            r2 = vecs[:, RT:2 * RT]
            hr = vecs[:, 2 * RT:3 * RT]
            nc.vector.scalar_tensor_tensor(
                out=sqerr_t[:], in0=q_t[:], scalar=-1.0, in1=h1x2,
                op0=mybir.AluOpType.mult, op1=mybir.AluOpType.add)
            nc.vector.tensor_tensor(out=sim_t[:], in0=q_t[:], in1=r2,
                                    op=mybir.AluOpType.mult)
            nc.vector.tensor_tensor(out=sim_t[:], in0=sim_t[:], in1=hr,
                                    op=mybir.AluOpType.subtract)
            nc.scalar.activation(out=exp_t[:], in_=sim_t[:],
                                 func=mybir.ActivationFunctionType.Exp,
                                 scale=-ALPHA)
            nc.vector.tensor_tensor(out=intra_acc[:], in0=sqerr_t[:],
                                    in1=exp_t[:], op=mybir.AluOpType.mult)

            nc.sync.dma_start(out=intra_dram.ap(), in_=intra_acc[:])
            nc.sync.dma_start(out=adv_dram.ap(), in_=adv_acc[:])

    nc.compile()
    return nc


def _get_nc():
    if "nc" not in _CACHE:
        _CACHE["nc"] = _build()
    return _CACHE["nc"]


def _make_in_maps(features, labels, centers):
    import ml_dtypes
    f8np = ml_dtypes.float8_e4m3
    bf16np = ml_dtypes.bfloat16

    features = np.ascontiguousarray(np.asarray(features, dtype=np.float32))
    labels = np.asarray(labels).astype(np.int64)
    centers = np.ascontiguousarray(np.asarray(centers, dtype=np.float32))

    # The loss is invariant to a batch permutation. Sort by label so
    # same-label pairs land within ~30 rows of the diagonal; the device then
    # needs only unmasked row sums plus two narrow corrections per row-tile.
    perm = np.argsort(labels, kind="stable")
    f = features[perm]
    labs = labels[perm]
    lab16 = labs.astype(np.float16)  # exact for values < 2048

    fnorm = np.sqrt((f.astype(np.float64) ** 2).sum(1))            # [B]
    cnorm_tab = np.sqrt((centers.astype(np.float64) ** 2).sum(1))  # [C]
    fhat8 = (f / np.maximum(fnorm, EPS)[:, None].astype(np.float32)
             ).astype(f8np)                                        # [B, D]
    cb = centers[labs]                                             # [B, D]
    cnorm = cnorm_tab[labs]                                        # [B]
    h1 = fnorm ** 2 + cnorm ** 2                                   # [B] f64
    rprod = 1.0 / (np.maximum(fnorm, EPS) * np.maximum(cnorm, EPS))
    s_bf = (f + cb).astype(bf16np)                                 # [B, D]

    in_maps = []
    for c in range(NCORES):
        s = c * SHARD
        rolled = (np.arange(LROWS) + s) % B
        # fhat_t[p, k*LROWS + c] = fhat[rolled[c], k*128 + p]
        v = fhat8[rolled]                          # [LROWS, D]
        fhat_t = np.ascontiguousarray(
            v.T.reshape(KT, 128, LROWS).transpose(1, 0, 2)
        ).reshape(128, KT * LROWS)
        # packed per-row scalars, laid out [partition, slot] exactly as the
        # SBUF tile wants them (vec[p, g*RT + t] = value for row t*128+p);
        # the last RT columns carry the row labels (exact in f32)
        vecs = np.empty((128, NVEC + RT), np.float32)
        own = slice(s, s + SHARD)
        for g, arr in enumerate((2.0 * h1, rprod / 2.0, h1 * rprod / 2.0,
                                 labs.astype(np.float64))):
            vecs[:, g * RT:(g + 1) * RT] = \
                arr[own].astype(np.float32).reshape(RT, 128).T
        in_maps.append({
            "fhat_t": fhat_t,
            "s_in": np.ascontiguousarray(s_bf[own]),
            "lab_f16": np.ascontiguousarray(lab16[rolled[:LABCOLS]]),
            "vecs": vecs,
        })
    return in_maps, labs


def _combine(results, labels):
    w = np.array(W_SLOT, dtype=np.float64)
    # per-slot sign: ACT slots hold +hinge, DVE slots hold -hinge
    # (min-form); corrections (13/14) follow j==0 which is pinned to DVE.
    sgn = np.empty((RT, SLOTS), dtype=np.float64)
    for t in range(RT):
        for j in range(NCH):
            s = 1.0 if ENG[(t, j)] == "a" else -1.0
            if j == 0:
                sgn[t, 0] = sgn[t, 1] = s
            elif j < NCH - 1:
                sgn[t, 1 + j] = s
            else:
                sgn[t, 11] = sgn[t, 12] = s
        sgn[t, 13] = sgn[t, 14] = -1.0
    hinge_total = 0.0
    intra_total = 0.0
    for c in range(NCORES):
        adv = results[c]["adv_out"].astype(np.float64).reshape(128, RT, SLOTS)
        hinge_total += float((adv.sum(axis=0) * sgn * w).sum())
        intra_total += float(results[c]["intra_out"].astype(np.float64).sum())
    cnt = np.bincount(labels, minlength=C).astype(np.float64)
    n_pairs = float(B) * B - float((cnt * cnt).sum())
    n_pairs = max(n_pairs, 1.0)
    loss = intra_total / B + LAMBDA_ADV * (hinge_total / n_pairs)
    return np.float32(loss)


def kernel(features, labels, centers):
    from concourse.bass_utils import run_bass_kernel_spmd
    nc = _get_nc()
    in_maps, labels64 = _make_in_maps(features, labels, centers)
    res = run_bass_kernel_spmd(nc, in_maps, core_ids=list(range(NCORES)))
    return _combine(res.results, labels64)
